# revision 1
# baseline (speedup 1.0000x reference)
"""Trainium2 Bass kernel for nn_LlamaMLPInfer (SwiGLU MLP, int4 group-quantized
weights), tensor-parallel over 8 NeuronCores.

Sharding: gate/up weights column-sharded over the intermediate dim (I/8 = 1792
rows per core), down weight row-sharded to match; every core sees the full
token stream (2*2048 = 4096 tokens). Per-core partial down outputs are
combined with an on-device ReduceScatter over the token dim, so core c ends
up with final output tokens [c*128 + b*1024, +128) per t-block b; the host
just concatenates shards (no host-side arithmetic on the output).

Weights are dequantized (q * group-scale) on the host and shipped as bf16 —
numerically identical to on-device bf16 dequant. Matmuls run in bf16 with
fp32 PSUM accumulation.
"""

import sys

if "/opt/trn_rl_repo" not in sys.path:
    sys.path.insert(0, "/opt/trn_rl_repo")

import numpy as np
import ml_dtypes

import concourse.mybir as mybir
import concourse.tile as tile
from concourse import bacc
from concourse.bass_utils import run_bass_kernel_spmd

BF16 = mybir.dt.bfloat16
F32 = mybir.dt.float32
BF = ml_dtypes.bfloat16

H = 4096           # hidden
I = 14336          # intermediate
G = 128            # quant group size (along the contraction dim everywhere)
T = 4096           # tokens (2 * 2048)
NCORES = 8
IC = I // NCORES   # 1792 intermediate rows per core
NIT = IC // 128    # 14 i-tiles per core
NHG = H // G       # 32 h-groups (gate/up contraction tiles)
TB = 1024          # token block
NBLK = T // TB     # 4
TS = 512           # t-subtile for gate/up psum (free dim)
HD2 = H // 2       # down-proj h half (2048)

_cache = {}


def _build():
    nc = bacc.Bacc("TRN2", target_bir_lowering=False, debug=False,
                   num_devices=NCORES)

    # DRAM inputs (per core).  wg/wu pre-packed on host as [it, p, g, i] so a
    # whole i-tile's weights are one contiguous 1 MiB DMA.
    xT = nc.dram_tensor("xT", [H, T], BF16, kind="ExternalInput")
    wg = nc.dram_tensor("wg", [NIT, 128, NHG, 128], BF16, kind="ExternalInput")
    wu = nc.dram_tensor("wu", [NIT, 128, NHG, 128], BF16, kind="ExternalInput")
    wd = nc.dram_tensor("wd", [IC, H], BF16, kind="ExternalInput")
    out = nc.dram_tensor("out", [NBLK, TB // NCORES, H], F32,
                         kind="ExternalOutput")

    with tile.TileContext(nc) as tc:
        with (
            tc.tile_pool(name="sbuf", bufs=1) as sbuf,
            tc.tile_pool(name="psum", bufs=1, space="PSUM") as psum,
            tc.tile_pool(name="dram", bufs=1, space="DRAM") as dram,
        ):
            for b in range(NBLK):
                # ---- x block: [128, g*TB + t] bf16, 64 KB/partition ----
                x_sb = sbuf.tile([128, NHG * TB], BF16, tag="xblk", bufs=1,
                                 name=f"x_sb_{b}")
                for g in range(NHG):
                    nc.sync.dma_start(
                        x_sb[:, g * TB:(g + 1) * TB],
                        xT[g * 128:(g + 1) * 128, b * TB:(b + 1) * TB])

                h_sb = sbuf.tile([128, NIT * TB], BF16, tag="hact", bufs=1,
                                 name=f"h_sb_{b}")

                # ---- gate/up projections + SwiGLU epilogue ----
                for it in range(NIT):
                    wg_sb = sbuf.tile([128, NHG * 128], BF16, tag="wg",
                                      bufs=2, name=f"wg_sb_{b}_{it}")
                    nc.sync.dma_start(wg_sb[:], wg[it])
                    wu_sb = sbuf.tile([128, NHG * 128], BF16, tag="wu",
                                      bufs=2, name=f"wu_sb_{b}_{it}")
                    nc.sync.dma_start(wu_sb[:], wu[it])

                    pg = [psum.tile([128, TS], F32, tag="pg", bufs=2,
                                    name=f"pg_{b}_{it}_{t}") for t in range(2)]
                    pu = [psum.tile([128, TS], F32, tag="pu", bufs=2,
                                    name=f"pu_{b}_{it}_{t}") for t in range(2)]
                    for g in range(NHG):
                        lhs_g = wg_sb[:, g * 128:(g + 1) * 128]
                        lhs_u = wu_sb[:, g * 128:(g + 1) * 128]
                        st, sp = (g == 0), (g == NHG - 1)
                        for t in range(2):
                            rhs = x_sb[:, g * TB + t * TS: g * TB + (t + 1) * TS]
                            nc.tensor.matmul(pg[t][:], lhs_g, rhs, start=st, stop=sp)
                            nc.tensor.matmul(pu[t][:], lhs_u, rhs, start=st, stop=sp)
                    for t in range(2):
                        sg = sbuf.tile([128, TS], BF16, tag="sg", bufs=3,
                                       name=f"sg_{b}_{it}_{t}")
                        nc.scalar.activation(sg[:], pg[t][:],
                                             mybir.ActivationFunctionType.Silu)
                        nc.vector.tensor_mul(
                            h_sb[:, it * TB + t * TS: it * TB + (t + 1) * TS],
                            sg[:], pu[t][:])

                # ---- down projection (partial over this core's I shard) ----
                rs_in = dram.tile([TB, H], F32, tag="rs_in", bufs=2,
                                  name=f"rs_in_{b}")
                for hh in range(2):
                    wd_sb = []
                    for ig in range(NIT):
                        w = sbuf.tile([128, HD2], BF16, tag=f"wd{ig}", bufs=1,
                                      name=f"wd_sb_{b}_{hh}_{ig}")
                        nc.sync.dma_start(
                            w[:], wd[ig * 128:(ig + 1) * 128,
                                     hh * HD2:(hh + 1) * HD2])
                        wd_sb.append(w)
                    for tsub in range(TB // 128):
                        pd = [psum.tile([128, 512], F32, tag="pd", bufs=4,
                                        name=f"pd_{b}_{hh}_{tsub}_{q}")
                              for q in range(4)]
                        for ig in range(NIT):
                            lhs = h_sb[:, ig * TB + tsub * 128:
                                       ig * TB + (tsub + 1) * 128]
                            st, sp = (ig == 0), (ig == NIT - 1)
                            for q in range(4):
                                nc.tensor.matmul(
                                    pd[q][:], lhs,
                                    wd_sb[ig][:, q * 512:(q + 1) * 512],
                                    start=st, stop=sp)
                        stage = sbuf.tile([128, HD2], F32, tag="stage", bufs=2,
                                          name=f"stage_{b}_{hh}_{tsub}")
                        for q in range(4):
                            nc.scalar.copy(stage[:, q * 512:(q + 1) * 512],
                                           pd[q][:])
                        nc.sync.dma_start(
                            rs_in[tsub * 128:(tsub + 1) * 128,
                                  hh * HD2:(hh + 1) * HD2],
                            stage[:])

                # ---- combine partial sums across cores ----
                rs_out = dram.tile([TB // NCORES, H], F32, tag="rs_out",
                                   bufs=2, name=f"rs_out_{b}")
                nc.gpsimd.collective_compute(
                    "ReduceScatter",
                    mybir.AluOpType.add,
                    replica_groups=[list(range(NCORES))],
                    ins=[rs_in.opt()],
                    outs=[rs_out.opt()],
                )
                nc.sync.dma_start(out[b], rs_out[:])

    nc.compile()
    return nc


def _prep_inputs(x, gate_q, gate_s, up_q, up_s, down_q, down_s):
    """Host-side shard + layout prep. Returns per-core input maps."""
    x = np.asarray(x)
    xT = np.ascontiguousarray(
        x.reshape(T, H).T).astype(BF)                      # [H, T]

    def dequant(q, s):
        q = np.asarray(q).astype(np.float32)
        s = np.asarray(s).astype(np.float32)
        o, i = q.shape
        return (q.reshape(o, i // G, G) * s[:, :, None]).reshape(o, i)

    Wg = dequant(gate_q, gate_s)     # [I, H] f32
    Wu = dequant(up_q, up_s)         # [I, H]
    Wd = dequant(down_q, down_s)     # [H, I]

    in_maps = []
    for c in range(NCORES):
        sl = slice(c * IC, (c + 1) * IC)
        # gate/up: [IC, H] -> transpose [H, IC] -> pack [it, p, g, i]
        def pack(w):
            wT = np.ascontiguousarray(w[sl].T).astype(BF)          # [H, IC]
            return np.ascontiguousarray(
                wT.reshape(NHG, 128, NIT, 128).transpose(2, 1, 0, 3))
        wdT = np.ascontiguousarray(Wd[:, sl].T).astype(BF)         # [IC, H]
        in_maps.append({
            "xT": xT,
            "wg": pack(Wg),
            "wu": pack(Wu),
            "wd": wdT,
        })
    return in_maps


def _assemble(results):
    """results[c]['out'] has shape [NBLK, 128, H]; token t = b*TB + c*128 + r."""
    out = np.empty((T, H), dtype=np.float32)
    for c in range(NCORES):
        o = results[c]["out"]
        for b in range(NBLK):
            t0 = b * TB + c * (TB // NCORES)
            out[t0:t0 + TB // NCORES] = o[b]
    return out.reshape(2, 2048, H)


def get_nc():
    if "nc" not in _cache:
        _cache["nc"] = _build()
    return _cache["nc"]


def kernel(x, gate_q, gate_s, up_q, up_s, down_q, down_s):
    nc = get_nc()
    in_maps = _prep_inputs(x, gate_q, gate_s, up_q, up_s, down_q, down_s)
    res = run_bass_kernel_spmd(nc, in_maps, core_ids=list(range(NCORES)))
    return _assemble(res.results)


# revision 10
# speedup vs baseline: 5.3392x; 5.3392x over previous
"""Trainium2 Bass kernel for nn_LlamaMLPInfer (SwiGLU MLP, int4 group-quantized
weights), tensor-parallel over 8 NeuronCores.

Sharding: gate/up weights column-sharded over the intermediate dim (I/8 = 1792
rows per core), down weight row-sharded to match; every core sees the full
token stream (2*2048 = 4096 tokens). Per-core partial down outputs are
combined with an on-device ReduceScatter over the token dim, so core c ends
up with final output tokens [b*1024 + c*128, +128) per t-block b; the host
just concatenates shards (no host-side arithmetic on the output).

Weights are dequantized (q * group-scale) on the host and shipped as bf16 —
numerically identical to on-device bf16 dequant. Matmuls run in bf16 with
fp32 PSUM accumulation.
"""

import sys

if "/opt/trn_rl_repo" not in sys.path:
    sys.path.insert(0, "/opt/trn_rl_repo")

import numpy as np
import ml_dtypes

import concourse.mybir as mybir
import concourse.tile as tile
from concourse import bacc
from concourse.bass_utils import run_bass_kernel_spmd

BF16 = mybir.dt.bfloat16
F32 = mybir.dt.float32
BF = ml_dtypes.bfloat16

H = 4096           # hidden
I = 14336          # intermediate
G = 128            # quant group size (along the contraction dim everywhere)
T = 4096           # tokens (2 * 2048)
NCORES = 8
IC = I // NCORES   # 1792 intermediate rows per core
NIT = IC // 128    # 14 i-tiles per core
NHG = H // G       # 32 h-groups (gate/up contraction tiles)
TB = 1024          # token block
NBLK = T // TB     # 4
TS = 512           # t-subtile for gate/up psum (free dim)
HD2 = H // 2       # down-proj h half (2048)

_cache = {}


def _emit_block(nc, sbuf, psum, dram, xT, wg, wu, wd, out, mode, rep, b):
    u = f"{rep}_{b}"

    # ---- x block: [128, g*TB + t] bf16, 64 KB/partition ----
    # 4 h-groups per DMA via 3D access pattern (8 DMAs instead of 32)
    x_sb = sbuf.tile([128, NHG * TB], BF16, tag="xblk", bufs=1,
                     name=f"x_sb_{u}")
    for g0 in range(0, NHG, 4):
        src = xT[g0 * 128:(g0 + 4) * 128, b * TB:(b + 1) * TB]
        nc.sync.dma_start(
            x_sb[:, g0 * TB:(g0 + 4) * TB].rearrange("p (f t) -> p f t", f=4),
            src.rearrange("(f p) t -> p f t", p=128))

    h_sb = sbuf.tile([128, NIT * TB], BF16, tag="hact", bufs=1,
                     name=f"h_sb_{u}")

    # ---- gate/up projections + SwiGLU epilogue ----
    for it in range(NIT):
        wg_sb = sbuf.tile([128, NHG * 128], BF16, tag="wg",
                          bufs=2, name=f"wg_sb_{u}_{it}")
        nc.sync.dma_start(wg_sb[:], wg[it])
        wu_sb = sbuf.tile([128, NHG * 128], BF16, tag="wu",
                          bufs=2, name=f"wu_sb_{u}_{it}")
        nc.sync.dma_start(wu_sb[:], wu[it])

        # One contiguous accumulation run per PSUM bank — alternating banks
        # MM-by-MM halves PE throughput (measured 210 vs 105 ns/MM).
        pg = [psum.tile([128, TS], F32, tag="ps", bufs=8,
                        name=f"pg_{u}_{it}_{t}") for t in range(2)]
        pu = [psum.tile([128, TS], F32, tag="ps", bufs=8,
                        name=f"pu_{u}_{it}_{t}") for t in range(2)]
        for w_sb, ps in ((wg_sb, pg), (wu_sb, pu)):
            for t in range(2):
                for g in range(NHG):
                    nc.tensor.matmul(
                        ps[t][:], w_sb[:, g * 128:(g + 1) * 128],
                        x_sb[:, g * TB + t * TS: g * TB + (t + 1) * TS],
                        start=(g == 0), stop=(g == NHG - 1))
        for t in range(2):
            sg = sbuf.tile([128, TS], BF16, tag="sg", bufs=3,
                           name=f"sg_{u}_{it}_{t}")
            nc.scalar.activation(sg[:], pg[t][:],
                                 mybir.ActivationFunctionType.Silu)
            nc.vector.tensor_mul(
                h_sb[:, it * TB + t * TS: it * TB + (t + 1) * TS],
                sg[:], pu[t][:])

    if mode == "gateup":
        # keep h_sb alive: tiny DMA out so nothing upstream is dead
        scratch = dram.tile([128, 128], BF16, tag="scr", bufs=2,
                            name=f"scr_{u}")
        nc.sync.dma_start(scratch[:], h_sb[:, :128])
        return

    # ---- down projection (partial over this core's I shard) ----
    # h split into quarters; one coalesced weight DMA per quarter,
    # double-buffered so the next quarter's load overlaps this quarter's
    # matmuls. Partials staged and reduce-scattered in bf16.
    HQ = H // 4  # 1024
    rs_in = dram.tile([TB, H], BF16, tag="rs_in", bufs=2, name=f"rs_in_{u}")
    for hq in range(4):
        wd_sb = sbuf.tile([128, NIT * HQ], BF16, tag="wd", bufs=2,
                          name=f"wd_sb_{u}_{hq}")
        src = wd[:, hq * HQ:(hq + 1) * HQ]
        nc.sync.dma_start(
            wd_sb[:].rearrange("p (ig h) -> p ig h", ig=NIT),
            src.rearrange("(ig p) h -> p ig h", p=128))
        for tsub in range(TB // 128):
            pd = [psum.tile([128, 512], F32, tag="ps", bufs=8,
                            name=f"pd_{u}_{hq}_{tsub}_{q}")
                  for q in range(2)]
            for q in range(2):
                for ig in range(NIT):
                    nc.tensor.matmul(
                        pd[q][:],
                        h_sb[:, ig * TB + tsub * 128: ig * TB + (tsub + 1) * 128],
                        wd_sb[:, ig * HQ + q * 512: ig * HQ + (q + 1) * 512],
                        start=(ig == 0), stop=(ig == NIT - 1))
            stage = sbuf.tile([128, HQ], BF16, tag="stage", bufs=4,
                              name=f"stage_{u}_{hq}_{tsub}")
            for q in range(2):
                nc.scalar.copy(stage[:, q * 512:(q + 1) * 512], pd[q][:])
            nc.gpsimd.dma_start(
                rs_in[tsub * 128:(tsub + 1) * 128, hq * HQ:(hq + 1) * HQ],
                stage[:])

    # ---- combine partial sums across cores ----
    if mode == "no_rs":
        rs_out = rs_in[:TB // NCORES, :]
    else:
        rs_out = dram.tile([TB // NCORES, H], BF16, tag="rs_out", bufs=2,
                           name=f"rs_out_{u}")
        nc.gpsimd.collective_compute(
            "ReduceScatter",
            mybir.AluOpType.add,
            replica_groups=[list(range(NCORES))],
            ins=[rs_in.opt()],
            outs=[rs_out.opt()],
        )
    # upcast the reduced bf16 shard to the f32 output in 1024-col chunks
    for ch in range(4):
        fo_b = sbuf.tile([128, HQ], BF16, tag="fo_b", bufs=2,
                         name=f"fo_b_{u}_{ch}")
        nc.sync.dma_start(fo_b[:], rs_out[:, ch * HQ:(ch + 1) * HQ])
        fo_f = sbuf.tile([128, HQ], F32, tag="fo_f", bufs=2,
                         name=f"fo_f_{u}_{ch}")
        nc.vector.tensor_copy(fo_f[:], fo_b[:])
        nc.sync.dma_start(out[b][:, ch * HQ:(ch + 1) * HQ], fo_f[:])


def _build(reps=1, mode="full"):
    """mode: 'full' | 'no_rs' (skip collective) | 'gateup' (skip down+rs)."""
    nc = bacc.Bacc("TRN2", target_bir_lowering=False, debug=False,
                   num_devices=NCORES)

    # DRAM inputs (per core).  wg/wu pre-packed on host as [it, p, g, i] so a
    # whole i-tile's weights are one contiguous 1 MiB DMA.
    xT = nc.dram_tensor("xT", [H, T], BF16, kind="ExternalInput")
    wg = nc.dram_tensor("wg", [NIT, 128, NHG, 128], BF16, kind="ExternalInput")
    wu = nc.dram_tensor("wu", [NIT, 128, NHG, 128], BF16, kind="ExternalInput")
    wd = nc.dram_tensor("wd", [IC, H], BF16, kind="ExternalInput")
    out = nc.dram_tensor("out", [NBLK, TB // NCORES, H], F32,
                         kind="ExternalOutput")

    with tile.TileContext(nc) as tc:
        with (
            tc.tile_pool(name="sbuf", bufs=1) as sbuf,
            tc.tile_pool(name="psum", bufs=1, space="PSUM") as psum,
            tc.tile_pool(name="dram", bufs=1, space="DRAM") as dram,
        ):
            for rep in range(reps):
                for b in range(NBLK):
                    _emit_block(nc, sbuf, psum, dram, xT, wg, wu, wd, out,
                                mode, rep, b)

    nc.compile()
    return nc


def _prep_inputs(x, gate_q, gate_s, up_q, up_s, down_q, down_s):
    """Host-side shard + layout prep. Returns per-core input maps."""
    x = np.asarray(x)
    xT = np.ascontiguousarray(x.reshape(T, H).T).astype(BF)        # [H, T]

    def dequant(q, s):
        q = np.asarray(q).astype(np.float32)
        s = np.asarray(s).astype(np.float32)
        o, i = q.shape
        return (q.reshape(o, i // G, G) * s[:, :, None]).reshape(o, i)

    Wg = dequant(gate_q, gate_s)     # [I, H] f32
    Wu = dequant(up_q, up_s)         # [I, H]
    Wd = dequant(down_q, down_s)     # [H, I]

    in_maps = []
    for c in range(NCORES):
        sl = slice(c * IC, (c + 1) * IC)

        def pack(w):
            # [IC, H] -> [H, IC] -> [it, p, g, i]
            wT = np.ascontiguousarray(w[sl].T).astype(BF)
            return np.ascontiguousarray(
                wT.reshape(NHG, 128, NIT, 128).transpose(2, 1, 0, 3))

        wdT = np.ascontiguousarray(Wd[:, sl].T).astype(BF)         # [IC, H]
        in_maps.append({
            "xT": xT,
            "wg": pack(Wg),
            "wu": pack(Wu),
            "wd": wdT,
        })
    return in_maps


def _assemble(results):
    """results[c]['out'] has shape [NBLK, 128, H]; token t = b*TB + c*128 + r."""
    out = np.empty((T, H), dtype=np.float32)
    tb8 = TB // NCORES
    for c in range(NCORES):
        o = results[c]["out"]
        for b in range(NBLK):
            t0 = b * TB + c * tb8
            out[t0:t0 + tb8] = o[b]
    return out.reshape(2, 2048, H)


def get_nc():
    if "nc" not in _cache:
        _cache["nc"] = _build()
    return _cache["nc"]


def kernel(x, gate_q, gate_s, up_q, up_s, down_q, down_s):
    nc = get_nc()
    in_maps = _prep_inputs(x, gate_q, gate_s, up_q, up_s, down_q, down_s)
    res = run_bass_kernel_spmd(nc, in_maps, core_ids=list(range(NCORES)))
    return _assemble(res.results)
